# revision 1
# baseline (speedup 1.0000x reference)
"""Trainium2 Bass kernel for token-level contrastive loss (CLIP-style with
softmax token pooling), distributed over 8 NeuronCores.

Strategy: shard the token axis T (196 -> padded 200 = 8 cores x 25 slices).
Each core computes, for its local t-slices, the full [B, B] partial softmax-
pooling sums S = sum_t exp(cos_t) and V = sum_t cos_t*exp(cos_t); these are
AllReduced, then every core redundantly computes the scalar bidirectional
InfoNCE loss (core 0's output is returned).

Per-core pipeline (per group of G=5 t-slices):
  - SWDGE cast-DMA loads fp32 [B, G, D] -> bf16 SBUF tiles
  - token norms via fused tensor_tensor_reduce (square + free-dim reduce)
  - visual tokens normalized in natural layout (per-partition scalar mul)
  - both operands transposed to [D, token] layout on TensorE (PE transpose)
  - per (b-tile, t): bf16 matmul -> dots PSUM; exp on ScalarE with
    per-partition scale folding in the text norm; cos*e via one fused
    scalar_tensor_tensor on DVE; S/V accumulated over t in PSUM by
    identity-matmuls; flushed to SBUF fp32 accumulators per group.
"""

import sys

sys.path.insert(0, "/opt/trn_rl_repo")

import numpy as np

import concourse.bass as bass
import concourse.mybir as mybir
import concourse.tile as tile
from concourse import bacc
from concourse.bass import ds, ts
from concourse.bass_utils import run_bass_kernel_spmd
from concourse.masks import make_identity

B = 512
T = 196
D = 256
NCORES = 8
TPAD = 200
TLOC = TPAD // NCORES  # 25
G = 5                  # t-slices per pipeline group
NG = TLOC // G         # 5 groups
NB = B // 128          # 4 b-tiles
NPAD = TPAD - T        # 4 zero pad slices globally
TEMP = 0.07

F32 = mybir.dt.float32
BF16 = mybir.dt.bfloat16

import os
_USE_AR = os.environ.get("K_NO_AR", "0") != "1"
_STOP_AFTER = int(os.environ.get("K_STOP_AFTER", "99"))  # debug: truncate stages


def _build_program():
    nc = bacc.Bacc(
        "TRN2",
        target_bir_lowering=False,
        debug=False,
        num_devices=NCORES,
    )
    text_in = nc.dram_tensor("text", [B, TLOC, D], F32, kind="ExternalInput")
    vis_in = nc.dram_tensor("vis", [B, TLOC, D], F32, kind="ExternalInput")
    out = nc.dram_tensor("out", [1, 1], F32, kind="ExternalOutput")

    # DRAM views grouped for partition-major loads: [p, i(b-tile), t, d]
    text_v = text_in.ap().rearrange("(i p) t d -> p i t d", p=128)
    vis_v = vis_in.ap().rearrange("(i p) t d -> p i t d", p=128)

    with tile.TileContext(nc) as tc:
        with (
            tc.tile_pool(name="const", bufs=1) as cpool,
            tc.tile_pool(name="raw", bufs=2) as rawpool,
            tc.tile_pool(name="tr", bufs=2) as trpool,
            tc.tile_pool(name="et", bufs=2 * G) as epool,
            tc.tile_pool(name="acc", bufs=1) as accpool,
            tc.tile_pool(name="fin", bufs=1) as finpool,
            tc.tile_pool(name="ps1", bufs=1, space="PSUM") as ps_tr,
            tc.tile_pool(name="ps2", bufs=2, space="PSUM") as ps_dots,
            tc.tile_pool(name="dram", bufs=1, space="DRAM") as dpool,
        ):
            ps_acc = ps_tr  # shares the bufs=1 PSUM pool
            # ---- constants ----
            ident = cpool.tile([128, 128], BF16, tag="ident")
            make_identity(nc, ident[:])
            ones = cpool.tile([128, 1], F32, tag="ones")
            nc.gpsimd.memset(ones[:], 1.0)
            eps_b = cpool.tile([128, 1], F32, tag="epsb")
            nc.gpsimd.memset(eps_b[:], 1e-12)
            diag_mask = cpool.tile([128, NB, 512], F32, tag="dmask")
            nc.gpsimd.memset(diag_mask[:], 0.0)
            # 1.0 where c - 128*i - p == 0  (c global col within b-tile i row block)
            nc.gpsimd.affine_select(
                out=diag_mask[:],
                in_=diag_mask[:],
                compare_op=mybir.AluOpType.not_equal,
                fill=1.0,
                base=0,
                pattern=[[-128, NB], [1, 512]],
                channel_multiplier=-1,
            )

            # ---- persistent accumulators / stats ----
            S_sb = accpool.tile([128, NB, 512], F32, tag="S")
            V_sb = accpool.tile([128, NB, 512], F32, tag="V")
            n2_t = accpool.tile([128, NB, TLOC], F32, tag="n2t")
            n2_v = accpool.tile([128, NB, TLOC], F32, tag="n2v")
            r_t = accpool.tile([128, NB, TLOC], F32, tag="rt")
            r_v = accpool.tile([128, NB, TLOC], F32, tag="rv")
            rs_scr = accpool.tile([128, NB, G], F32, tag="rs_scr")
            sq_scr = accpool.tile([128, 256], BF16, tag="sq")

            for g in range(NG if _STOP_AFTER >= 1 else 0):
                tg = ds(g * G, G)
                # ---- loads (SWDGE cast fp32 -> bf16) ----
                rt_raw = rawpool.tile([128, NB, G, D], BF16, tag="rtext")
                rv_raw = rawpool.tile([128, NB, G, D], BF16, tag="rvis")
                nc.gpsimd.dma_start(rt_raw[:], text_v[:, :, tg, :])
                nc.gpsimd.dma_start(rv_raw[:], vis_v[:, :, tg, :])
                if _STOP_AFTER < 2:
                    continue

                # ---- token norms (fused square+reduce), per (tensor, i, t) ----
                for i in range(NB):
                    for t in range(G):
                        tt = g * G + t
                        nc.vector.scalar_tensor_tensor(
                            out=sq_scr[:],
                            in0=rt_raw[:, i, t, :],
                            scalar=1.0,
                            in1=rt_raw[:, i, t, :],
                            op0=mybir.AluOpType.mult,
                            op1=mybir.AluOpType.mult,
                            accum_out=n2_t[:, i, ds(tt, 1)],
                        )
                        nc.vector.scalar_tensor_tensor(
                            out=sq_scr[:],
                            in0=rv_raw[:, i, t, :],
                            scalar=1.0,
                            in1=rv_raw[:, i, t, :],
                            op0=mybir.AluOpType.mult,
                            op1=mybir.AluOpType.mult,
                            accum_out=n2_v[:, i, ds(tt, 1)],
                        )

                # r = 1/sqrt(n2 + 1e-12) = exp(-0.5*ln(n2 + eps)); both on ACT
                # (bias guards pad zeros; Ln/Exp share one table set)
                for n2, r in ((n2_t, r_t), (n2_v, r_v)):
                    nc.scalar.activation(
                        rs_scr[:],
                        n2[:, :, tg],
                        mybir.ActivationFunctionType.Ln,
                        bias=eps_b[:],
                    )
                    nc.scalar.activation(
                        r[:, :, tg],
                        rs_scr[:],
                        mybir.ActivationFunctionType.Exp,
                        scale=-0.5,
                    )

                # ---- normalize visual in place (per-partition scalar) ----
                for i in range(NB):
                    for t in range(G):
                        tt = g * G + t
                        nc.vector.tensor_scalar_mul(
                            rv_raw[:, i, t, :],
                            rv_raw[:, i, t, :],
                            r_v[:, i, ds(tt, 1)],
                        )

                if _STOP_AFTER < 3:
                    continue
                # ---- PE transposes to [d, token] + PSUM->SBUF copies ----
                texT = trpool.tile([128, 2, G, 512], BF16, tag="texT")
                visT = trpool.tile([128, 2, G, 512], BF16, tag="visT")
                for src, dstT, tag in ((rt_raw, texT, "ptT"), (rv_raw, visT, "ptV")):
                    for h in range(2):
                        for t in range(G):
                            p_tr = ps_tr.tile([128, 512], BF16, tag=f"{tag}{h}")
                            for i in range(NB):
                                nc.tensor.transpose(
                                    p_tr[:, ts(i, 128)],
                                    src[:, i, t, ds(h * 128, 128)],
                                    ident[:],
                                )
                            nc.scalar.copy(dstT[:, h, t, :], p_tr[:])

                if _STOP_AFTER < 4:
                    continue
                # ---- per (b-tile, t): dots, exp, cos*e, PSUM accumulation ----
                for i in range(NB):
                    S_ps = ps_acc.tile([128, 512], F32, tag="Sps")
                    V_ps = ps_acc.tile([128, 512], F32, tag="Vps")
                    es = []
                    tmps = []
                    for t in range(G):
                        tt = g * G + t
                        dots = ps_dots.tile([128, 512], F32, tag="dots")
                        nc.tensor.matmul(
                            dots[:], texT[:, 0, t, ts(i, 128)], visT[:, 0, t, :],
                            start=True, stop=False,
                        )
                        nc.tensor.matmul(
                            dots[:], texT[:, 1, t, ts(i, 128)], visT[:, 1, t, :],
                            start=False, stop=True,
                        )
                        e_t = epool.tile([128, 512], BF16, tag="e")
                        nc.scalar.activation(
                            e_t[:], dots[:],
                            mybir.ActivationFunctionType.Exp,
                            scale=r_t[:, i, ds(tt, 1)],
                        )
                        tmp_t = epool.tile([128, 512], BF16, tag="tmp")
                        # tmp = (dots * r_b) * e = cos * e
                        nc.vector.scalar_tensor_tensor(
                            out=tmp_t[:],
                            in0=dots[:],
                            scalar=r_t[:, i, ds(tt, 1)],
                            in1=e_t[:],
                            op0=mybir.AluOpType.mult,
                            op1=mybir.AluOpType.mult,
                        )
                        es.append(e_t)
                        tmps.append(tmp_t)
                    for t in range(G):
                        nc.tensor.matmul(
                            S_ps[:], ident[:], es[t][:],
                            start=(t == 0), stop=(t == G - 1),
                        )
                    for t in range(G):
                        nc.tensor.matmul(
                            V_ps[:], ident[:], tmps[t][:],
                            start=(t == 0), stop=(t == G - 1),
                        )
                    # flush into fp32 SBUF accumulators
                    if g == 0:
                        nc.vector.tensor_copy(S_sb[:, i, :], S_ps[:])
                        nc.vector.tensor_copy(V_sb[:, i, :], V_ps[:])
                    else:
                        nc.vector.tensor_add(S_sb[:, i, :], S_sb[:, i, :], S_ps[:])
                        nc.vector.tensor_add(V_sb[:, i, :], V_sb[:, i, :], V_ps[:])

            def _tail():
                # ---- AllReduce S and V across the 8 cores ----
                cc_in = dpool.tile([2, 128, NB * 512], F32, tag="cc_in")
                cc_out = dpool.tile(
                    [2, 128, NB * 512], F32, tag="cc_out", addr_space="Shared"
                )
                nc.sync.dma_start(cc_in[0], S_sb[:].rearrange("p i c -> p (i c)"))
                nc.sync.dma_start(cc_in[1], V_sb[:].rearrange("p i c -> p (i c)"))
                if _USE_AR:
                    nc.gpsimd.collective_compute(
                        "AllReduce",
                        mybir.AluOpType.add,
                        replica_groups=[list(range(NCORES))],
                        ins=[cc_in[:].opt()],
                        outs=[cc_out[:].opt()],
                    )
                else:
                    nc.sync.dma_start(cc_out[:], cc_in[:])
                nc.sync.dma_start(S_sb[:].rearrange("p i c -> p (i c)"), cc_out[0])
                nc.sync.dma_start(V_sb[:].rearrange("p i c -> p (i c)"), cc_out[1])

                # ---- final scalar loss (computed redundantly on every core) ----
                # pad correction: each global pad slice added exp(0)=1 to S
                nc.vector.tensor_scalar_add(S_sb[:], S_sb[:], float(-NPAD))
                sim = finpool.tile([128, NB, 512], F32, tag="sim")
                scr = finpool.tile([128, NB, 512], F32, tag="scr")
                # sim = V / S via 1/S = exp(-ln(S))  (S > 0 always)
                nc.scalar.activation(
                    scr[:], S_sb[:], mybir.ActivationFunctionType.Ln
                )
                nc.scalar.activation(
                    scr[:], scr[:], mybir.ActivationFunctionType.Exp, scale=-1.0
                )
                nc.vector.tensor_mul(sim[:], V_sb[:], scr[:])
                # diag sum per partition (over the 4 b-tiles): sum(sim * mask)
                diag_p = finpool.tile([128, 1], F32, tag="diagp")
                nc.vector.scalar_tensor_tensor(
                    out=scr[:],
                    in0=sim[:],
                    scalar=1.0,
                    in1=diag_mask[:],
                    op0=mybir.AluOpType.mult,
                    op1=mybir.AluOpType.mult,
                    accum_out=diag_p[:],
                )
                # e2 = exp(sim/TEMP), rowsums fused per b-tile
                e2 = finpool.tile([128, NB, 512], F32, tag="e2")
                rowsum = finpool.tile([128, NB], F32, tag="rowsum")
                for i in range(NB):
                    nc.scalar.activation(
                        e2[:, i, :], sim[:, i, :],
                        mybir.ActivationFunctionType.Exp,
                        scale=1.0 / TEMP,
                        accum_out=rowsum[:, ds(i, 1)],
                    )
                lse_row = finpool.tile([128, NB], F32, tag="lserow")
                nc.scalar.activation(
                    lse_row[:], rowsum[:], mybir.ActivationFunctionType.Ln,
                )
                # column sums over all b (partition dim) via ones-matmuls
                col_ps = ps_tr.tile([1, 512], F32, tag="Sps")
                for i in range(NB):
                    nc.tensor.matmul(
                        col_ps[:], ones[:], e2[:, i, :],
                        start=(i == 0), stop=(i == NB - 1),
                    )
                lse_col = finpool.tile([1, 512], F32, tag="lsecol")
                nc.scalar.activation(
                    lse_col[:], col_ps[:], mybir.ActivationFunctionType.Ln,
                )
                csum = finpool.tile([1, 1], F32, tag="csum")
                nc.vector.reduce_sum(csum[:], lse_col[:], axis=mybir.AxisListType.X)
                # row-lse partition sum and diag partition sum via ones-matmuls
                red_ps = ps_tr.tile([1, 2], F32, tag="Vps")
                lse_row_red = finpool.tile([128, 1], F32, tag="lserr")
                nc.vector.reduce_sum(
                    lse_row_red[:], lse_row[:], axis=mybir.AxisListType.X
                )
                nc.tensor.matmul(
                    red_ps[:, 0:1], ones[:], lse_row_red[:], start=True, stop=True
                )
                nc.tensor.matmul(
                    red_ps[:, 1:2], ones[:], diag_p[:], start=True, stop=True
                )
                red_sb = finpool.tile([1, 2], F32, tag="redsb")
                nc.vector.tensor_copy(red_sb[:], red_ps[:])
                # loss = 0.5*(rsum + csum)/B - diag/(B*TEMP)
                t_a = finpool.tile([1, 1], F32, tag="ta")
                nc.vector.tensor_add(t_a[:], red_sb[:, 0:1], csum[:])
                nc.vector.tensor_scalar_mul(t_a[:], t_a[:], 0.5 / B)
                t_b = finpool.tile([1, 1], F32, tag="tb")
                nc.vector.tensor_scalar_mul(
                    t_b[:], red_sb[:, 1:2], 1.0 / (B * TEMP)
                )
                loss_t = finpool.tile([1, 1], F32, tag="loss")
                nc.vector.tensor_sub(loss_t[:], t_a[:], t_b[:])
                nc.sync.dma_start(out.ap(), loss_t[:])

            if _STOP_AFTER < 4:
                dbg = finpool.tile([1, 1], F32, tag="dbg")
                nc.gpsimd.memset(dbg[:], 42.0)
                nc.sync.dma_start(out.ap(), dbg[:])
            else:
                _tail()

    nc.compile()
    return nc


_CACHE = {}


def _get_program():
    if "nc" not in _CACHE:
        _CACHE["nc"] = _build_program()
    return _CACHE["nc"]


def kernel(text_tokens: np.ndarray, visual_tokens: np.ndarray) -> np.ndarray:
    text = np.ascontiguousarray(np.asarray(text_tokens, dtype=np.float32))
    vis = np.ascontiguousarray(np.asarray(visual_tokens, dtype=np.float32))
    assert text.shape == (B, T, D) and vis.shape == (B, T, D)

    tp = np.zeros((B, TPAD, D), np.float32)
    vp = np.zeros((B, TPAD, D), np.float32)
    tp[:, :T] = text
    vp[:, :T] = vis

    in_maps = []
    for k in range(NCORES):
        sl = slice(k * TLOC, (k + 1) * TLOC)
        in_maps.append(
            {
                "text": np.ascontiguousarray(tp[:, sl]),
                "vis": np.ascontiguousarray(vp[:, sl]),
            }
        )

    nc = _get_program()
    res = run_bass_kernel_spmd(nc, in_maps, core_ids=list(range(NCORES)))
    loss = np.float32(res.results[0]["out"].reshape(-1)[0])
    return np.asarray(loss, dtype=np.float32).reshape(())

